# revision 1
# baseline (speedup 1.0000x reference)
"""AdaptiveGraphConvolution on 8 TRN2 NeuronCores.

Math: out = sum_l m_l * segment_sum_l(val * (x @ W_l) gathered by col) + bias
Reordered: aggregate in input-feature space first (per graph), project after:
    g_l[r, :] = sum_{e in graph l, row_e = r} val_e * x[col_e, :]
    out[r, :] = sum_l g_l[r, :] @ (m_l * W_l) + bias

Sharding: destination rows across 8 cores (6250 rows each). Per core,
dest rows processed in 49 blocks of 128 rows. Edges of a block are packed
into 128-edge chunks (graph-pure). Per chunk:
  - dma_gather fetches the 128 source rows x[col] (bf16, 256B each) from HBM
  - a host-prebuilt "assignment" matrix A [128 edge, 128 destrow] bf16 with
    A[e, loc_e] = val_e is streamed from HBM
  - TensorE: gT_psum[l] += G_chunk^T @ A_chunk   ([feat, row] accumulation)
Then per block: ACT copies gT psum->SBUF (bf16), TensorE projects
out3 += gT_l^T @ W'_l (row-major), DVE adds bias, sync DMA stores.

idx trick: gather indices are int16; cols up to 49999 exceed 32767, so the
gather base is x[32768] and idx = col - 32768 (hardware treats idx as signed;
verified on silicon).
"""

import math
import numpy as np
import ml_dtypes

N_NODES = 50000
N_GRAPHS = 4
N_EDGES = 800000
D = 128
N_CORES = 8
ROWS_PER_CORE = N_NODES // N_CORES  # 6250
BLOCK = 128
NB = math.ceil(ROWS_PER_CORE / BLOCK)  # 49
NBUF = 2  # G/A double buffering


def _host_schedule(edge_rows, edge_cols, edge_vals):
    """Build the SPMD-uniform chunk schedule + per-core idx/A arrays."""
    rows = np.asarray(edge_rows).astype(np.int64).ravel()  # [4*800000] graph-major
    cols = np.asarray(edge_cols).astype(np.int64).ravel()
    vals = np.asarray(edge_vals, dtype=np.float32).ravel()
    graph = np.repeat(np.arange(N_GRAPHS, dtype=np.int64), N_EDGES)

    core = rows // ROWS_PER_CORE
    local = rows - core * ROWS_PER_CORE
    blk = local // BLOCK
    loc = local % BLOCK

    # group key: (core, block, graph); count per group
    gkey = (core * NB + blk) * N_GRAPHS + graph
    n_groups = N_CORES * NB * N_GRAPHS
    cnt = np.bincount(gkey, minlength=n_groups).reshape(N_CORES, NB, N_GRAPHS)

    # uniform chunk counts across cores
    C = np.maximum(1, np.ceil(cnt.max(axis=0) / 128).astype(np.int64))  # [NB, 4]
    C_b = C.sum(axis=1)  # chunks per block
    chunk_base = np.zeros((NB, N_GRAPHS), dtype=np.int64)
    flat = C.ravel()
    chunk_base.ravel()[1:] = np.cumsum(flat)[:-1]
    total_chunks = int(flat.sum())

    # rank of each edge within its (core, block, graph) group
    order = np.argsort(gkey, kind="stable")
    sorted_key = gkey[order]
    grp_start = np.searchsorted(sorted_key, np.arange(n_groups), side="left")
    rank_sorted = np.arange(len(order)) - grp_start[sorted_key]
    rank = np.empty_like(rank_sorted)
    rank[order] = rank_sorted

    chunk_in_run = rank // 128
    slot = rank % 128
    chunk = chunk_base[blk, graph] + chunk_in_run  # global chunk id (per core stream)
    pos = chunk * 128 + slot  # position in the core's edge stream

    total_idx = total_chunks * 128
    idx_arrs, a_arrs = [], []
    for s in range(N_CORES):
        m = core == s
        idx_flat = np.zeros(total_idx, dtype=np.int16)
        idx_flat[pos[m]] = (cols[m] - 32768).astype(np.int16)
        wrapped = idx_flat.reshape(-1, 16).T  # [16, total_idx/16]
        idx_arrs.append(np.tile(wrapped, (8, 1)).copy())

        A = np.zeros((128, total_chunks, 128), dtype=ml_dtypes.bfloat16)
        A[slot[m], chunk[m], loc[m]] = vals[m].astype(ml_dtypes.bfloat16)
        a_arrs.append(A.reshape(128, total_chunks * 128))

    return {
        "C": C,
        "C_b": C_b,
        "total_chunks": total_chunks,
        "idx_arrs": idx_arrs,
        "a_arrs": a_arrs,
    }


def _build_nc(C, C_b, total_chunks):
    import concourse.bacc as bacc
    import concourse.bass as bass
    import concourse.mybir as mybir
    from concourse.library_config import mlp
    import contextlib

    Cmax = int(C_b.max())
    total8 = total_chunks * 8
    NBl = NB
    # offsets per block, in chunks
    off = np.zeros(NBl + 1, dtype=np.int64)
    off[1:] = np.cumsum(C_b)
    row_cnt = [min(BLOCK, ROWS_PER_CORE - BLOCK * b) for b in range(NBl)]

    nc = bacc.Bacc("TRN2", dynamic_dma_scratch_size=32768)
    bf16 = mybir.dt.bfloat16
    f32 = mybir.dt.float32

    x_d = nc.declare_dram_parameter("x", [N_NODES, D], bf16, isOutput=False)
    idx_d = nc.declare_dram_parameter("idxs", [128, total8], mybir.dt.int16, isOutput=False)
    a_d = nc.declare_dram_parameter("amat", [128, total_chunks * 128], bf16, isOutput=False)
    wp_d = nc.declare_dram_parameter("wp", [128, N_GRAPHS * D], bf16, isOutput=False)
    bias_d = nc.declare_dram_parameter("biasr", [128, D], f32, isOutput=False)
    out_d = nc.declare_dram_parameter("out", [ROWS_PER_CORE, D], f32, isOutput=True)

    with contextlib.ExitStack() as ctx:
        block = ctx.enter_context(nc.Block())
        idx_sb = ctx.enter_context(nc.sbuf_tensor("idx_sb", [128, total8], mybir.dt.int16))
        g_bufs = [
            ctx.enter_context(nc.sbuf_tensor(f"g{i}", [128, Cmax, D], bf16))
            for i in range(NBUF)
        ]
        a_bufs = [
            ctx.enter_context(nc.sbuf_tensor(f"a{i}", [128, Cmax * 128], bf16))
            for i in range(NBUF)
        ]
        wp_sb = ctx.enter_context(nc.sbuf_tensor("wp_sb", [128, N_GRAPHS * D], bf16))
        bias_sb = ctx.enter_context(nc.sbuf_tensor("bias_sb", [128, D], f32))
        gt_sb = ctx.enter_context(nc.sbuf_tensor("gt_sb", [128, 2 * N_GRAPHS * D], bf16))
        stage = ctx.enter_context(nc.sbuf_tensor("stage", [128, 2 * D], f32))
        gt_ps = [
            ctx.enter_context(nc.psum_tensor(f"gt{i}", [128, N_GRAPHS, D], f32))
            for i in range(2)
        ]
        o3_ps = [
            ctx.enter_context(nc.psum_tensor(f"o3{i}", [128, D], f32)) for i in range(2)
        ]
        io = ctx.enter_context(nc.semaphore("io"))
        a_sem = ctx.enter_context(nc.semaphore("a_sem"))
        gather_sem = ctx.enter_context(nc.semaphore("gather_sem"))
        store_sem = ctx.enter_context(nc.semaphore("store_sem"))
        pe_g = ctx.enter_context(nc.semaphore("pe_g"))
        pe_proj = ctx.enter_context(nc.semaphore("pe_proj"))
        act_sem = ctx.enter_context(nc.semaphore("act_sem"))
        dve_sem = ctx.enter_context(nc.semaphore("dve_sem"))

        @block.sync
        def _(sync):
            sync.dma_start(idx_sb[:, :], idx_d[:, :]).then_inc(io, 16)
            sync.dma_start(wp_sb[:, :], wp_d[:, :]).then_inc(io, 16)
            sync.dma_start(bias_sb[:, :], bias_d[:, :]).then_inc(io, 16)
            for b in range(NBl):
                cb = int(C_b[b])
                if b >= NBUF:
                    # A buffer reuse: PE done with block b-NBUF
                    sync.wait_ge(pe_g, 4 * (b - NBUF) + 4)
                sync.dma_start(
                    a_bufs[b % NBUF][:, : cb * 128],
                    a_d[:, int(off[b]) * 128 : int(off[b] + cb) * 128],
                ).then_inc(a_sem, 16)
                if b >= 2:
                    sb = b - 2  # store block b-2
                    sync.wait_ge(dve_sem, sb + 1)
                    sync.dma_start(
                        out_d[BLOCK * sb : BLOCK * sb + row_cnt[sb], :],
                        stage[: row_cnt[sb], (sb % 2) * D : (sb % 2) * D + D],
                    ).then_inc(store_sem, 16)
            for sb in (NBl - 2, NBl - 1):
                sync.wait_ge(dve_sem, sb + 1)
                sync.dma_start(
                    out_d[BLOCK * sb : BLOCK * sb + row_cnt[sb], :],
                    stage[: row_cnt[sb], (sb % 2) * D : (sb % 2) * D + D],
                ).then_inc(store_sem, 16)

        @block.gpsimd
        def _(gpsimd):
            gpsimd.load_library(mlp)
            gpsimd.wait_ge(io, 16)  # idx array resident (first io DMA)
            for b in range(NBl):
                cb = int(C_b[b])
                if b >= NBUF:
                    gpsimd.wait_ge(pe_g, 4 * (b - NBUF) + 4)
                gpsimd.dma_gather(
                    g_bufs[b % NBUF][:, :cb, :],
                    x_d[32768:, :],
                    idx_sb[:, int(off[b]) * 8 : int(off[b] + cb) * 8],
                    cb * 128,
                    cb * 128,
                    D,
                    single_packet=False,
                ).then_inc(gather_sem, 16)

        @block.tensor
        def _(tensor):
            tensor.wait_ge(io, 48)
            for b in range(NBl):
                tensor.wait_ge(gather_sem, 16 * (b + 1))
                tensor.wait_ge(a_sem, 16 * (b + 1))
                if b >= 2:
                    tensor.wait_ge(dve_sem, b - 1)  # o3 psum reuse
                gbuf = g_bufs[b % NBUF]
                abuf = a_bufs[b % NBUF]
                ci = 0
                for l in range(N_GRAPHS):
                    cl = int(C[b, l])
                    for i in range(cl):
                        mm = tensor.matmul(
                            gt_ps[b % 2][:, l, :],
                            gbuf[:, ci, :],
                            abuf[:, ci * 128 : (ci + 1) * 128],
                            start=(i == 0),
                            stop=(i == cl - 1),
                        )
                        ci += 1
                    mm.then_inc(pe_g, 1)
                for l in range(N_GRAPHS):
                    tensor.wait_ge(act_sem, 4 * b + l + 1)
                    tensor.matmul(
                        o3_ps[b % 2][:, :],
                        gt_sb[:, ((b % 2) * N_GRAPHS + l) * D : ((b % 2) * N_GRAPHS + l + 1) * D],
                        wp_sb[:, l * D : (l + 1) * D],
                        start=(l == 0),
                        stop=(l == N_GRAPHS - 1),
                    ).then_inc(pe_proj, 1)

        @block.scalar
        def _(scalar):
            for b in range(NBl):
                for l in range(N_GRAPHS):
                    scalar.wait_ge(pe_g, 4 * b + 4)  # whole gt bank written
                    if b >= 2:
                        scalar.wait_ge(pe_proj, 4 * (b - 2) + l + 1)  # gt_sb reuse
                    scalar.copy(
                        gt_sb[:, ((b % 2) * N_GRAPHS + l) * D : ((b % 2) * N_GRAPHS + l + 1) * D],
                        gt_ps[b % 2][:, l, :],
                    ).then_inc(act_sem, 1)

        @block.vector
        def _(vector):
            for b in range(NBl):
                vector.wait_ge(pe_proj, 4 * b + 4)
                if b >= 2:
                    vector.wait_ge(store_sem, 16 * (b - 1))  # stage reuse
                vector.tensor_add(
                    stage[:, (b % 2) * D : (b % 2) * D + D],
                    o3_ps[b % 2][:, :],
                    bias_sb[:, :],
                ).then_inc(dve_sem, 1)

    nc.compile()
    return nc


_TRACE = {"on": False, "last": None}


def kernel(x, edge_rows, edge_cols, edge_vals, W, mixing_weight, bias):
    from concourse.bass_utils import run_bass_kernel_spmd

    sched = _host_schedule(edge_rows, edge_cols, edge_vals)
    nc = _build_nc(sched["C"], sched["C_b"], sched["total_chunks"])

    x_bf16 = np.asarray(x, dtype=np.float32).astype(ml_dtypes.bfloat16)
    Wp = (np.asarray(mixing_weight, dtype=np.float32)[:, 0, None, None]
          * np.asarray(W, dtype=np.float32))  # [4,128,128]
    wp_arr = np.ascontiguousarray(
        np.transpose(Wp, (1, 0, 2)).reshape(D, N_GRAPHS * D)
    ).astype(ml_dtypes.bfloat16)
    bias_rep = np.ascontiguousarray(
        np.broadcast_to(np.asarray(bias, dtype=np.float32), (128, D))
    )

    in_maps = [
        {
            "x": x_bf16,
            "idxs": sched["idx_arrs"][s],
            "amat": sched["a_arrs"][s],
            "wp": wp_arr,
            "biasr": bias_rep,
        }
        for s in range(N_CORES)
    ]

    res = run_bass_kernel_spmd(
        nc, in_maps, core_ids=list(range(N_CORES)), trace=_TRACE["on"]
    )
    _TRACE["last"] = res
    out = np.concatenate(
        [np.asarray(res.results[s]["out"], dtype=np.float32) for s in range(N_CORES)],
        axis=0,
    )
    return out


# revision 2
# speedup vs baseline: 1.0463x; 1.0463x over previous
"""AdaptiveGraphConvolution on 8 TRN2 NeuronCores.

Math: out = sum_l m_l * segment_sum_l(val * (x @ W_l) gathered by col) + bias
Reordered: aggregate in input-feature space first (per graph), project after:
    g_l[r, :] = sum_{e in graph l, row_e = r} val_e * x[col_e, :]
    out[r, :] = sum_l g_l[r, :] @ (m_l * W_l) + bias

Sharding: destination rows across 8 cores (6250 rows each). Per core,
dest rows processed in 49 blocks of 128 rows. Edges of a block are packed
into 128-edge chunks (graph-pure). Per chunk:
  - dma_gather fetches the 128 source rows x[col] (bf16, 256B each) from HBM
  - a host-prebuilt "assignment" matrix A [128 edge, 128 destrow] bf16 with
    A[e, loc_e] = val_e is streamed from HBM
  - TensorE: gT_psum[l] += G_chunk^T @ A_chunk   ([feat, row] accumulation)
Then per block: ACT copies gT psum->SBUF (bf16), TensorE projects
out3 += gT_l^T @ W'_l (row-major), DVE adds bias, sync DMA stores.

idx trick: gather indices are int16; cols up to 49999 exceed 32767, so the
gather base is x[32768] and idx = col - 32768 (hardware treats idx as signed;
verified on silicon).
"""

import math
import numpy as np
import ml_dtypes

N_NODES = 50000
N_GRAPHS = 4
N_EDGES = 800000
D = 128
N_CORES = 8
ROWS_PER_CORE = N_NODES // N_CORES  # 6250
BLOCK = 128
NB = math.ceil(ROWS_PER_CORE / BLOCK)  # 49
NBUF = 2  # G double buffering
NBUF_A = 3  # A-slab prefetch depth


def _host_schedule(edge_rows, edge_cols, edge_vals):
    """Build the SPMD-uniform chunk schedule + per-core idx/A arrays."""
    rows = np.asarray(edge_rows).astype(np.int64).ravel()  # [4*800000] graph-major
    cols = np.asarray(edge_cols).astype(np.int64).ravel()
    vals = np.asarray(edge_vals, dtype=np.float32).ravel()
    graph = np.repeat(np.arange(N_GRAPHS, dtype=np.int64), N_EDGES)

    core = rows // ROWS_PER_CORE
    local = rows - core * ROWS_PER_CORE
    blk = local // BLOCK
    loc = local % BLOCK

    # group key: (core, block, graph); count per group
    gkey = (core * NB + blk) * N_GRAPHS + graph
    n_groups = N_CORES * NB * N_GRAPHS
    cnt = np.bincount(gkey, minlength=n_groups).reshape(N_CORES, NB, N_GRAPHS)

    # uniform chunk counts across cores
    C = np.maximum(1, np.ceil(cnt.max(axis=0) / 128).astype(np.int64))  # [NB, 4]
    C_b = C.sum(axis=1)  # chunks per block
    chunk_base = np.zeros((NB, N_GRAPHS), dtype=np.int64)
    flat = C.ravel()
    chunk_base.ravel()[1:] = np.cumsum(flat)[:-1]
    total_chunks = int(flat.sum())

    # rank of each edge within its (core, block, graph) group
    order = np.argsort(gkey, kind="stable")
    sorted_key = gkey[order]
    grp_start = np.searchsorted(sorted_key, np.arange(n_groups), side="left")
    rank_sorted = np.arange(len(order)) - grp_start[sorted_key]
    rank = np.empty_like(rank_sorted)
    rank[order] = rank_sorted

    chunk_in_run = rank // 128
    slot = rank % 128
    chunk = chunk_base[blk, graph] + chunk_in_run  # global chunk id (per core stream)
    pos = chunk * 128 + slot  # position in the core's edge stream

    total_idx = total_chunks * 128
    idx_arrs, a_arrs = [], []
    for s in range(N_CORES):
        m = core == s
        idx_flat = np.zeros(total_idx, dtype=np.int16)
        idx_flat[pos[m]] = (cols[m] - 32768).astype(np.int16)
        wrapped = idx_flat.reshape(-1, 16).T  # [16, total_idx/16]
        idx_arrs.append(np.tile(wrapped, (8, 1)).copy())

        A = np.zeros((128, total_chunks, 128), dtype=ml_dtypes.bfloat16)
        A[slot[m], chunk[m], loc[m]] = vals[m].astype(ml_dtypes.bfloat16)
        a_arrs.append(A.reshape(128, total_chunks * 128))

    return {
        "C": C,
        "C_b": C_b,
        "total_chunks": total_chunks,
        "idx_arrs": idx_arrs,
        "a_arrs": a_arrs,
    }


def _build_nc(C, C_b, total_chunks):
    import concourse.bacc as bacc
    import concourse.bass as bass
    import concourse.mybir as mybir
    from concourse.library_config import mlp
    import contextlib

    Cmax = int(C_b.max())
    total8 = total_chunks * 8
    NBl = NB
    # offsets per block, in chunks
    off = np.zeros(NBl + 1, dtype=np.int64)
    off[1:] = np.cumsum(C_b)
    row_cnt = [min(BLOCK, ROWS_PER_CORE - BLOCK * b) for b in range(NBl)]

    nc = bacc.Bacc("TRN2", dynamic_dma_scratch_size=32768)
    bf16 = mybir.dt.bfloat16
    f32 = mybir.dt.float32

    x_d = nc.declare_dram_parameter("x", [N_NODES, D], bf16, isOutput=False)
    idx_d = nc.declare_dram_parameter("idxs", [128, total8], mybir.dt.int16, isOutput=False)
    a_d = nc.declare_dram_parameter("amat", [128, total_chunks * 128], bf16, isOutput=False)
    wp_d = nc.declare_dram_parameter("wp", [128, N_GRAPHS * D], bf16, isOutput=False)
    bias_d = nc.declare_dram_parameter("biasr", [128, D], f32, isOutput=False)
    out_d = nc.declare_dram_parameter("out", [ROWS_PER_CORE, D], f32, isOutput=True)

    with contextlib.ExitStack() as ctx:
        block = ctx.enter_context(nc.Block())
        idx_sb = ctx.enter_context(nc.sbuf_tensor("idx_sb", [128, total8], mybir.dt.int16))
        g_bufs = [
            ctx.enter_context(nc.sbuf_tensor(f"g{i}", [128, Cmax, D], bf16))
            for i in range(NBUF)
        ]
        a_bufs = [
            ctx.enter_context(nc.sbuf_tensor(f"a{i}", [128, Cmax * 128], bf16))
            for i in range(NBUF_A)
        ]
        wp_sb = ctx.enter_context(nc.sbuf_tensor("wp_sb", [128, N_GRAPHS * D], bf16))
        bias_sb = ctx.enter_context(nc.sbuf_tensor("bias_sb", [128, D], f32))
        gt_sb = ctx.enter_context(nc.sbuf_tensor("gt_sb", [128, 2 * N_GRAPHS * D], bf16))
        stage = ctx.enter_context(nc.sbuf_tensor("stage", [128, 2 * D], f32))
        gt_ps = [
            ctx.enter_context(nc.psum_tensor(f"gt{i}", [128, N_GRAPHS, D], f32))
            for i in range(2)
        ]
        o3_ps = [
            ctx.enter_context(nc.psum_tensor(f"o3{i}", [128, D], f32)) for i in range(2)
        ]
        io = ctx.enter_context(nc.semaphore("io"))
        a_sem = ctx.enter_context(nc.semaphore("a_sem"))
        gather_sem = ctx.enter_context(nc.semaphore("gather_sem"))
        store_sem = ctx.enter_context(nc.semaphore("store_sem"))
        pe_g = ctx.enter_context(nc.semaphore("pe_g"))
        pe_proj = ctx.enter_context(nc.semaphore("pe_proj"))
        act_sem = ctx.enter_context(nc.semaphore("act_sem"))
        dve_sem = ctx.enter_context(nc.semaphore("dve_sem"))

        @block.sync
        def _(sync):
            sync.dma_start(idx_sb[:, :], idx_d[:, :]).then_inc(io, 16)
            sync.dma_start(wp_sb[:, :], wp_d[:, :]).then_inc(io, 16)
            sync.dma_start(bias_sb[:, :], bias_d[:, :]).then_inc(io, 16)
            for b in range(NBl):
                cb = int(C_b[b])
                if b >= NBUF_A:
                    # A buffer reuse: PE done with block b-NBUF_A
                    sync.wait_ge(pe_g, 4 * (b - NBUF_A) + 4)
                sync.dma_start(
                    a_bufs[b % NBUF_A][:, : cb * 128],
                    a_d[:, int(off[b]) * 128 : int(off[b] + cb) * 128],
                ).then_inc(a_sem, 16)
                if b >= 2:
                    sb = b - 2  # store block b-2
                    sync.wait_ge(dve_sem, sb + 1)
                    sync.dma_start(
                        out_d[BLOCK * sb : BLOCK * sb + row_cnt[sb], :],
                        stage[: row_cnt[sb], (sb % 2) * D : (sb % 2) * D + D],
                    ).then_inc(store_sem, 16)
            for sb in (NBl - 2, NBl - 1):
                sync.wait_ge(dve_sem, sb + 1)
                sync.dma_start(
                    out_d[BLOCK * sb : BLOCK * sb + row_cnt[sb], :],
                    stage[: row_cnt[sb], (sb % 2) * D : (sb % 2) * D + D],
                ).then_inc(store_sem, 16)

        @block.gpsimd
        def _(gpsimd):
            gpsimd.load_library(mlp)
            gpsimd.wait_ge(io, 16)  # idx array resident (first io DMA)
            for b in range(NBl):
                cb = int(C_b[b])
                if b >= NBUF:
                    gpsimd.wait_ge(pe_g, 4 * (b - NBUF) + 4)
                gpsimd.dma_gather(
                    g_bufs[b % NBUF][:, :cb, :],
                    x_d[32768:, :],
                    idx_sb[:, int(off[b]) * 8 : int(off[b] + cb) * 8],
                    cb * 128,
                    cb * 128,
                    D,
                    single_packet=False,
                ).then_inc(gather_sem, 16)

        @block.tensor
        def _(tensor):
            tensor.wait_ge(io, 48)
            for b in range(NBl):
                tensor.wait_ge(gather_sem, 16 * (b + 1))
                tensor.wait_ge(a_sem, 16 * (b + 1))
                if b >= 2:
                    tensor.wait_ge(dve_sem, b - 1)  # o3 psum reuse
                gbuf = g_bufs[b % NBUF]
                abuf = a_bufs[b % NBUF_A]
                ci = 0
                for l in range(N_GRAPHS):
                    cl = int(C[b, l])
                    for i in range(cl):
                        mm = tensor.matmul(
                            gt_ps[b % 2][:, l, :],
                            gbuf[:, ci, :],
                            abuf[:, ci * 128 : (ci + 1) * 128],
                            start=(i == 0),
                            stop=(i == cl - 1),
                        )
                        ci += 1
                    mm.then_inc(pe_g, 1)
                for l in range(N_GRAPHS):
                    tensor.wait_ge(act_sem, 4 * b + l + 1)
                    tensor.matmul(
                        o3_ps[b % 2][:, :],
                        gt_sb[:, ((b % 2) * N_GRAPHS + l) * D : ((b % 2) * N_GRAPHS + l + 1) * D],
                        wp_sb[:, l * D : (l + 1) * D],
                        start=(l == 0),
                        stop=(l == N_GRAPHS - 1),
                    ).then_inc(pe_proj, 1)

        @block.scalar
        def _(scalar):
            for b in range(NBl):
                for l in range(N_GRAPHS):
                    scalar.wait_ge(pe_g, 4 * b + 4)  # whole gt bank written
                    if b >= 2:
                        scalar.wait_ge(pe_proj, 4 * (b - 2) + l + 1)  # gt_sb reuse
                    scalar.copy(
                        gt_sb[:, ((b % 2) * N_GRAPHS + l) * D : ((b % 2) * N_GRAPHS + l + 1) * D],
                        gt_ps[b % 2][:, l, :],
                    ).then_inc(act_sem, 1)

        @block.vector
        def _(vector):
            for b in range(NBl):
                vector.wait_ge(pe_proj, 4 * b + 4)
                if b >= 2:
                    vector.wait_ge(store_sem, 16 * (b - 1))  # stage reuse
                vector.tensor_add(
                    stage[:, (b % 2) * D : (b % 2) * D + D],
                    o3_ps[b % 2][:, :],
                    bias_sb[:, :],
                ).then_inc(dve_sem, 1)

    nc.compile()
    return nc


_TRACE = {"on": False, "last": None}


def kernel(x, edge_rows, edge_cols, edge_vals, W, mixing_weight, bias):
    from concourse.bass_utils import run_bass_kernel_spmd

    sched = _host_schedule(edge_rows, edge_cols, edge_vals)
    nc = _build_nc(sched["C"], sched["C_b"], sched["total_chunks"])

    x_bf16 = np.asarray(x, dtype=np.float32).astype(ml_dtypes.bfloat16)
    Wp = (np.asarray(mixing_weight, dtype=np.float32)[:, 0, None, None]
          * np.asarray(W, dtype=np.float32))  # [4,128,128]
    wp_arr = np.ascontiguousarray(
        np.transpose(Wp, (1, 0, 2)).reshape(D, N_GRAPHS * D)
    ).astype(ml_dtypes.bfloat16)
    bias_rep = np.ascontiguousarray(
        np.broadcast_to(np.asarray(bias, dtype=np.float32), (128, D))
    )

    in_maps = [
        {
            "x": x_bf16,
            "idxs": sched["idx_arrs"][s],
            "amat": sched["a_arrs"][s],
            "wp": wp_arr,
            "biasr": bias_rep,
        }
        for s in range(N_CORES)
    ]

    res = run_bass_kernel_spmd(
        nc, in_maps, core_ids=list(range(N_CORES)), trace=_TRACE["on"]
    )
    _TRACE["last"] = res
    out = np.concatenate(
        [np.asarray(res.results[s]["out"], dtype=np.float32) for s in range(N_CORES)],
        axis=0,
    )
    return out


# revision 10
# speedup vs baseline: 1.1907x; 1.1380x over previous
"""AdaptiveGraphConvolution on 8 TRN2 NeuronCores.

Math: out = sum_l m_l * segment_sum_l(val * (x @ W_l) gathered by col) + bias
Reordered: aggregate in input-feature space first (per graph), project after:
    g_l[r, :] = sum_{e in graph l, row_e = r} val_e * x[col_e, :]
    out[r, :] = sum_l g_l[r, :] @ (m_l * W_l) + bias

Sharding: destination rows across 8 cores (6250 rows each). Per core,
dest rows processed in 49 blocks of 128 rows. Edges of a block are packed
into 128-edge chunks (graph-pure). Per chunk:
  - dma_gather fetches the 128 source rows x[col] (bf16, 256B each) from HBM
  - a host-prebuilt "assignment" matrix A [128 edge, 128 destrow] bf16 with
    A[e, loc_e] = val_e is streamed from HBM
  - TensorE: gT_psum[l] += G_chunk^T @ A_chunk   ([feat, row] accumulation)
Then per block: ACT copies gT psum->SBUF (bf16), TensorE projects
out3 += gT_l^T @ W'_l (row-major), DVE adds bias, sync DMA stores.

idx trick: gather indices are int16; cols up to 49999 exceed 32767, so the
gather base is x[32768] and idx = col - 32768 (hardware treats idx as signed;
verified on silicon).
"""

import math
import numpy as np
import ml_dtypes

N_NODES = 50000
N_GRAPHS = 4
N_EDGES = 800000
D = 128
N_CORES = 8
ROWS_PER_CORE = N_NODES // N_CORES  # 6250
BLOCK = 128
NB = math.ceil(ROWS_PER_CORE / BLOCK)  # 49
NBUF = 2  # G double buffering
NBUF_A = 3  # A-slab prefetch depth


def _host_schedule(edge_rows, edge_cols, edge_vals):
    """Build the SPMD-uniform chunk schedule + per-core idx/A arrays."""
    rows = np.asarray(edge_rows).astype(np.int64).ravel()  # [4*800000] graph-major
    cols = np.asarray(edge_cols).astype(np.int64).ravel()
    vals = np.asarray(edge_vals, dtype=np.float32).ravel()
    graph = np.repeat(np.arange(N_GRAPHS, dtype=np.int64), N_EDGES)

    core = rows // ROWS_PER_CORE
    local = rows - core * ROWS_PER_CORE
    blk = local // BLOCK
    loc = local % BLOCK

    # group key: (core, block, graph); count per group
    gkey = (core * NB + blk) * N_GRAPHS + graph
    n_groups = N_CORES * NB * N_GRAPHS
    cnt = np.bincount(gkey, minlength=n_groups).reshape(N_CORES, NB, N_GRAPHS)

    # uniform chunk counts across cores
    C = np.maximum(1, np.ceil(cnt.max(axis=0) / 128).astype(np.int64))  # [NB, 4]
    C_b = C.sum(axis=1)  # chunks per block
    total_chunks = int(C.sum())
    off = np.zeros(NB + 1, dtype=np.int64)
    off[1:] = np.cumsum(C_b)

    # Round-robin chunk order within each block: (l0 j0, l1 j0, ..., l0 j1, ...)
    # so cores' fill (low j first) concentrates padding at the call tail.
    Jmax = int(C.max())
    L = np.full((NB, N_GRAPHS, Jmax), -1, dtype=np.int64)  # (b,l,j) -> rr pos in block
    for b in range(NB):
        p = 0
        for j in range(int(C[b].max())):
            for l in range(N_GRAPHS):
                if j < C[b, l]:
                    L[b, l, j] = p
                    p += 1

    # rank of each edge within its (core, block, graph) group
    order = np.argsort(gkey, kind="stable")
    sorted_key = gkey[order]
    grp_start = np.searchsorted(sorted_key, np.arange(n_groups), side="left")
    rank_sorted = np.arange(len(order)) - grp_start[sorted_key]
    rank = np.empty_like(rank_sorted)
    rank[order] = rank_sorted

    chunk_in_run = rank // 128
    slot = rank % 128
    chunk = off[blk] + L[blk, graph, chunk_in_run]  # global chunk id (rr order)
    pos = chunk * 128 + slot  # position in the core's edge stream

    # per-call transferred idx count: cover every core's last real edge
    pos_in_call = pos - off[blk] * 128
    ni = np.zeros(NB, dtype=np.int64)
    np.maximum.at(ni, blk, pos_in_call + 1)
    ni = np.minimum(((ni + 15) // 16) * 16, C_b * 128).astype(np.int64)

    total_idx = total_chunks * 128
    idx_arrs, a_arrs = [], []
    for s in range(N_CORES):
        m = core == s
        idx_flat = np.zeros(total_idx, dtype=np.int16)
        idx_flat[pos[m]] = (cols[m] - 32768).astype(np.int16)
        wrapped = idx_flat.reshape(-1, 16).T  # [16, total_idx/16]
        idx_arrs.append(np.tile(wrapped, (8, 1)).copy())

        A = np.zeros((128, total_chunks, 128), dtype=ml_dtypes.bfloat16)
        A[slot[m], chunk[m], loc[m]] = vals[m].astype(ml_dtypes.bfloat16)
        a_arrs.append(A.reshape(128, total_chunks * 128))

    return {
        "C": C,
        "C_b": C_b,
        "L": L,
        "ni": ni,
        "total_chunks": total_chunks,
        "idx_arrs": idx_arrs,
        "a_arrs": a_arrs,
    }


def _build_nc(C, C_b, total_chunks, L, ni):
    import concourse.bacc as bacc
    import concourse.bass as bass
    import concourse.mybir as mybir
    from concourse.library_config import mlp
    import contextlib

    Cmax = int(C_b.max())
    total8 = total_chunks * 8
    NBl = NB
    # offsets per block, in chunks
    off = np.zeros(NBl + 1, dtype=np.int64)
    off[1:] = np.cumsum(C_b)
    row_cnt = [min(BLOCK, ROWS_PER_CORE - BLOCK * b) for b in range(NBl)]

    nc = bacc.Bacc("TRN2", dynamic_dma_scratch_size=32768)
    bf16 = mybir.dt.bfloat16
    f32 = mybir.dt.float32

    x_d = nc.declare_dram_parameter("x", [N_NODES, D], bf16, isOutput=False)
    idx_d = nc.declare_dram_parameter("idxs", [128, total8], mybir.dt.int16, isOutput=False)
    a_d = nc.declare_dram_parameter("amat", [128, total_chunks * 128], bf16, isOutput=False)
    wp_d = nc.declare_dram_parameter("wp", [128, N_GRAPHS * D], bf16, isOutput=False)
    bias_d = nc.declare_dram_parameter("biasr", [128, D], f32, isOutput=False)
    out_d = nc.declare_dram_parameter("out", [ROWS_PER_CORE, D], f32, isOutput=True)

    with contextlib.ExitStack() as ctx:
        block = ctx.enter_context(nc.Block())
        idx_sb = ctx.enter_context(nc.sbuf_tensor("idx_sb", [128, total8], mybir.dt.int16))
        g_bufs = [
            ctx.enter_context(nc.sbuf_tensor(f"g{i}", [128, Cmax, D], bf16))
            for i in range(NBUF)
        ]
        a_bufs = [
            ctx.enter_context(nc.sbuf_tensor(f"a{i}", [128, Cmax * 128], bf16))
            for i in range(NBUF_A)
        ]
        wp_sb = ctx.enter_context(nc.sbuf_tensor("wp_sb", [128, N_GRAPHS * D], bf16))
        bias_sb = ctx.enter_context(nc.sbuf_tensor("bias_sb", [128, D], f32))
        gt_sb = ctx.enter_context(nc.sbuf_tensor("gt_sb", [128, 2 * N_GRAPHS * D], bf16))
        stage = ctx.enter_context(nc.sbuf_tensor("stage", [128, 2 * D], f32))
        gt_ps = [
            ctx.enter_context(nc.psum_tensor(f"gt{i}", [128, N_GRAPHS, D], f32))
            for i in range(2)
        ]
        o3_ps = [
            ctx.enter_context(nc.psum_tensor(f"o3{i}", [128, D], f32)) for i in range(2)
        ]
        init_sem = ctx.enter_context(nc.semaphore("init_sem"))
        io = ctx.enter_context(nc.semaphore("io"))
        a_sem = ctx.enter_context(nc.semaphore("a_sem"))
        gather_sem = ctx.enter_context(nc.semaphore("gather_sem"))
        store_sem = ctx.enter_context(nc.semaphore("store_sem"))
        pe_g = ctx.enter_context(nc.semaphore("pe_g"))
        pe_proj = ctx.enter_context(nc.semaphore("pe_proj"))
        act_sem = ctx.enter_context(nc.semaphore("act_sem"))
        dve_sem = ctx.enter_context(nc.semaphore("dve_sem"))

        @block.sync
        def _(sync):
            sync.dma_start(idx_sb[:, :], idx_d[:, :]).then_inc(io, 16)
            sync.dma_start(wp_sb[:, :], wp_d[:, :]).then_inc(io, 16)
            sync.dma_start(bias_sb[:, :], bias_d[:, :]).then_inc(io, 16)
            for b in range(NBl):
                cb = int(C_b[b])
                if b >= NBUF_A:
                    # A buffer reuse: PE done with block b-NBUF_A
                    sync.wait_ge(pe_g, 4 * (b - NBUF_A) + 4)
                sync.dma_start(
                    a_bufs[b % NBUF_A][:, : cb * 128],
                    a_d[:, int(off[b]) * 128 : int(off[b] + cb) * 128],
                ).then_inc(a_sem, 16)
                if b >= 2:
                    sb = b - 2  # store block b-2
                    sync.wait_ge(dve_sem, sb + 1)
                    sync.dma_start(
                        out_d[BLOCK * sb : BLOCK * sb + row_cnt[sb], :],
                        stage[: row_cnt[sb], (sb % 2) * D : (sb % 2) * D + D],
                    ).then_inc(store_sem, 16)
            for sb in (NBl - 2, NBl - 1):
                sync.wait_ge(dve_sem, sb + 1)
                sync.dma_start(
                    out_d[BLOCK * sb : BLOCK * sb + row_cnt[sb], :],
                    stage[: row_cnt[sb], (sb % 2) * D : (sb % 2) * D + D],
                ).then_inc(store_sem, 16)

        @block.gpsimd
        def _(gpsimd):
            gpsimd.load_library(mlp)
            gpsimd.wait_ge(io, 16)  # idx array resident (first io DMA)
            gpsimd.wait_ge(init_sem, NBUF)  # G buffers zeroed
            for b in range(NBl):
                nib = int(ni[b])
                nslots = (nib + 127) // 128
                if b >= NBUF:
                    gpsimd.wait_ge(pe_g, 4 * (b - NBUF) + 4)
                gpsimd.dma_gather(
                    g_bufs[b % NBUF][:, :nslots, :],
                    x_d[32768:, :],
                    idx_sb[:, int(off[b]) * 8 : int(off[b]) * 8 + nslots * 8],
                    nslots * 128,
                    nib,
                    D,
                    single_packet=False,
                ).then_inc(gather_sem, 16)

        @block.tensor
        def _(tensor):
            tensor.wait_ge(io, 48)
            for b in range(NBl):
                tensor.wait_ge(gather_sem, 16 * (b + 1))
                tensor.wait_ge(a_sem, 16 * (b + 1))
                if b >= 2:
                    tensor.wait_ge(dve_sem, b - 1)  # o3 psum reuse
                gbuf = g_bufs[b % NBUF]
                abuf = a_bufs[b % NBUF_A]
                for l in range(N_GRAPHS):
                    cl = int(C[b, l])
                    for i in range(cl):
                        ci = int(L[b, l, i])
                        mm = tensor.matmul(
                            gt_ps[b % 2][:, l, :],
                            gbuf[:, ci, :],
                            abuf[:, ci * 128 : (ci + 1) * 128],
                            start=(i == 0),
                            stop=(i == cl - 1),
                        )
                    mm.then_inc(pe_g, 1)
                for l in range(N_GRAPHS):
                    tensor.wait_ge(act_sem, 4 * b + l + 1)
                    tensor.matmul(
                        o3_ps[b % 2][:, :],
                        gt_sb[:, ((b % 2) * N_GRAPHS + l) * D : ((b % 2) * N_GRAPHS + l + 1) * D],
                        wp_sb[:, l * D : (l + 1) * D],
                        start=(l == 0),
                        stop=(l == N_GRAPHS - 1),
                    ).then_inc(pe_proj, 1)

        @block.scalar
        def _(scalar):
            for i in range(NBUF):
                scalar.memzero(g_bufs[i][:, :, :]).then_inc(init_sem, 1)
            for b in range(NBl):
                for l in range(N_GRAPHS):
                    scalar.wait_ge(pe_g, 4 * b + 4)  # whole gt bank written
                    if b >= 2:
                        scalar.wait_ge(pe_proj, 4 * (b - 2) + l + 1)  # gt_sb reuse
                    scalar.copy(
                        gt_sb[:, ((b % 2) * N_GRAPHS + l) * D : ((b % 2) * N_GRAPHS + l + 1) * D],
                        gt_ps[b % 2][:, l, :],
                    ).then_inc(act_sem, 1)

        @block.vector
        def _(vector):
            for b in range(NBl):
                vector.wait_ge(pe_proj, 4 * b + 4)
                if b >= 2:
                    vector.wait_ge(store_sem, 16 * (b - 1))  # stage reuse
                vector.tensor_add(
                    stage[:, (b % 2) * D : (b % 2) * D + D],
                    o3_ps[b % 2][:, :],
                    bias_sb[:, :],
                ).then_inc(dve_sem, 1)

    nc.compile()
    return nc


_TRACE = {"on": False, "last": None}


def kernel(x, edge_rows, edge_cols, edge_vals, W, mixing_weight, bias):
    from concourse.bass_utils import run_bass_kernel_spmd

    sched = _host_schedule(edge_rows, edge_cols, edge_vals)
    nc = _build_nc(sched["C"], sched["C_b"], sched["total_chunks"], sched["L"], sched["ni"])

    x_bf16 = np.asarray(x, dtype=np.float32).astype(ml_dtypes.bfloat16)
    Wp = (np.asarray(mixing_weight, dtype=np.float32)[:, 0, None, None]
          * np.asarray(W, dtype=np.float32))  # [4,128,128]
    wp_arr = np.ascontiguousarray(
        np.transpose(Wp, (1, 0, 2)).reshape(D, N_GRAPHS * D)
    ).astype(ml_dtypes.bfloat16)
    bias_rep = np.ascontiguousarray(
        np.broadcast_to(np.asarray(bias, dtype=np.float32), (128, D))
    )

    in_maps = [
        {
            "x": x_bf16,
            "idxs": sched["idx_arrs"][s],
            "amat": sched["a_arrs"][s],
            "wp": wp_arr,
            "biasr": bias_rep,
        }
        for s in range(N_CORES)
    ]

    res = run_bass_kernel_spmd(
        nc, in_maps, core_ids=list(range(N_CORES)), trace=_TRACE["on"]
    )
    _TRACE["last"] = res
    out = np.concatenate(
        [np.asarray(res.results[s]["out"], dtype=np.float32) for s in range(N_CORES)],
        axis=0,
    )
    return out


# revision 11
# speedup vs baseline: 2.7506x; 2.3100x over previous
"""AdaptiveGraphConvolution on 8 TRN2 NeuronCores.

Math: out = sum_l m_l * segment_sum_l(val * (x @ W_l) gathered by col) + bias
Reordered: aggregate in input-feature space first (per graph), project after:
    g_l[r, :] = sum_{e in graph l, row_e = r} val_e * x[col_e, :]
    out[r, :] = sum_l g_l[r, :] @ (m_l * W_l) + bias

Sharding: destination rows across 8 cores (6250 rows each). Per core,
dest rows processed in 49 blocks of 128 rows. Edges of a block are packed
into 128-edge chunks (graph-pure). Per chunk:
  - dma_gather fetches the 128 source rows x[col] (bf16, 256B each) from HBM
  - a host-prebuilt "assignment" matrix A [128 edge, 128 destrow] bf16 with
    A[e, loc_e] = val_e is streamed from HBM
  - TensorE: gT_psum[l] += G_chunk^T @ A_chunk   ([feat, row] accumulation)
Then per block: ACT copies gT psum->SBUF (bf16), TensorE projects
out3 += gT_l^T @ W'_l (row-major), DVE adds bias, sync DMA stores.

idx trick: gather indices are int16; cols up to 49999 exceed 32767, so the
gather base is x[32768] and idx = col - 32768 (hardware treats idx as signed;
verified on silicon).
"""

import math
import numpy as np
import ml_dtypes

N_NODES = 50000
N_GRAPHS = 4
N_EDGES = 800000
D = 128
N_CORES = 8
ROWS_PER_CORE = N_NODES // N_CORES  # 6250
BLOCK = 128
NB = math.ceil(ROWS_PER_CORE / BLOCK)  # 49
NBUF = 3  # G buffering (>= in-flight gathers for multi-queue gen overlap)
NBUF_A = 3  # A-slab prefetch depth


def _host_schedule(edge_rows, edge_cols, edge_vals):
    """Build the SPMD-uniform chunk schedule + per-core idx/A arrays."""
    rows = np.asarray(edge_rows).astype(np.int64).ravel()  # [4*800000] graph-major
    cols = np.asarray(edge_cols).astype(np.int64).ravel()
    vals = np.asarray(edge_vals, dtype=np.float32).ravel()
    graph = np.repeat(np.arange(N_GRAPHS, dtype=np.int64), N_EDGES)

    core = rows // ROWS_PER_CORE
    local = rows - core * ROWS_PER_CORE
    blk = local // BLOCK
    loc = local % BLOCK

    # group key: (core, block, graph); count per group
    gkey = (core * NB + blk) * N_GRAPHS + graph
    n_groups = N_CORES * NB * N_GRAPHS
    cnt = np.bincount(gkey, minlength=n_groups).reshape(N_CORES, NB, N_GRAPHS)

    # uniform chunk counts across cores
    C = np.maximum(1, np.ceil(cnt.max(axis=0) / 128).astype(np.int64))  # [NB, 4]
    C_b = C.sum(axis=1)  # chunks per block
    total_chunks = int(C.sum())
    off = np.zeros(NB + 1, dtype=np.int64)
    off[1:] = np.cumsum(C_b)

    # Round-robin chunk order within each block: (l0 j0, l1 j0, ..., l0 j1, ...)
    # so cores' fill (low j first) concentrates padding at the call tail.
    Jmax = int(C.max())
    L = np.full((NB, N_GRAPHS, Jmax), -1, dtype=np.int64)  # (b,l,j) -> rr pos in block
    for b in range(NB):
        p = 0
        for j in range(int(C[b].max())):
            for l in range(N_GRAPHS):
                if j < C[b, l]:
                    L[b, l, j] = p
                    p += 1

    # rank of each edge within its (core, block, graph) group
    order = np.argsort(gkey, kind="stable")
    sorted_key = gkey[order]
    grp_start = np.searchsorted(sorted_key, np.arange(n_groups), side="left")
    rank_sorted = np.arange(len(order)) - grp_start[sorted_key]
    rank = np.empty_like(rank_sorted)
    rank[order] = rank_sorted

    chunk_in_run = rank // 128
    slot = rank % 128
    chunk = off[blk] + L[blk, graph, chunk_in_run]  # global chunk id (rr order)
    pos = chunk * 128 + slot  # position in the core's edge stream

    # per-call transferred idx count: cover every core's last real edge
    pos_in_call = pos - off[blk] * 128
    ni = np.zeros(NB, dtype=np.int64)
    np.maximum.at(ni, blk, pos_in_call + 1)
    ni = np.minimum(((ni + 15) // 16) * 16, C_b * 128).astype(np.int64)

    total_idx = total_chunks * 128
    idx_arrs, a_arrs = [], []
    for s in range(N_CORES):
        m = core == s
        idx_flat = np.zeros(total_idx, dtype=np.int16)
        idx_flat[pos[m]] = (cols[m] - 32768).astype(np.int16)
        wrapped = idx_flat.reshape(-1, 16).T  # [16, total_idx/16]
        idx_arrs.append(np.tile(wrapped, (8, 1)).copy())

        A = np.zeros((128, total_chunks, 128), dtype=ml_dtypes.bfloat16)
        A[slot[m], chunk[m], loc[m]] = vals[m].astype(ml_dtypes.bfloat16)
        a_arrs.append(A.reshape(128, total_chunks * 128))

    return {
        "C": C,
        "C_b": C_b,
        "L": L,
        "ni": ni,
        "total_chunks": total_chunks,
        "idx_arrs": idx_arrs,
        "a_arrs": a_arrs,
    }


def _build_nc(C, C_b, total_chunks, L, ni):
    import concourse.bacc as bacc
    import concourse.bass as bass
    import concourse.mybir as mybir
    from concourse.library_config import mlp
    import contextlib

    Cmax = int(C_b.max())
    total8 = total_chunks * 8
    NBl = NB
    # offsets per block, in chunks
    off = np.zeros(NBl + 1, dtype=np.int64)
    off[1:] = np.cumsum(C_b)
    row_cnt = [min(BLOCK, ROWS_PER_CORE - BLOCK * b) for b in range(NBl)]

    nc = bacc.Bacc("TRN2", dynamic_dma_scratch_size=32768, num_swdge_queues=4)
    bf16 = mybir.dt.bfloat16
    f32 = mybir.dt.float32

    x_d = nc.declare_dram_parameter("x", [N_NODES, D], bf16, isOutput=False)
    idx_d = nc.declare_dram_parameter("idxs", [128, total8], mybir.dt.int16, isOutput=False)
    a_d = nc.declare_dram_parameter("amat", [128, total_chunks * 128], bf16, isOutput=False)
    wp_d = nc.declare_dram_parameter("wp", [128, N_GRAPHS * D], bf16, isOutput=False)
    bias_d = nc.declare_dram_parameter("biasr", [128, D], f32, isOutput=False)
    out_d = nc.declare_dram_parameter("out", [ROWS_PER_CORE, D], f32, isOutput=True)

    with contextlib.ExitStack() as ctx:
        block = ctx.enter_context(nc.Block())
        idx_sb = ctx.enter_context(nc.sbuf_tensor("idx_sb", [128, total8], mybir.dt.int16))
        g_bufs = [
            ctx.enter_context(nc.sbuf_tensor(f"g{i}", [128, Cmax, D], bf16))
            for i in range(NBUF)
        ]
        a_bufs = [
            ctx.enter_context(nc.sbuf_tensor(f"a{i}", [128, Cmax * 128], bf16))
            for i in range(NBUF_A)
        ]
        wp_sb = ctx.enter_context(nc.sbuf_tensor("wp_sb", [128, N_GRAPHS * D], bf16))
        bias_sb = ctx.enter_context(nc.sbuf_tensor("bias_sb", [128, D], f32))
        gt_sb = ctx.enter_context(nc.sbuf_tensor("gt_sb", [128, 2 * N_GRAPHS * D], bf16))
        stage = ctx.enter_context(nc.sbuf_tensor("stage", [128, 2 * D], f32))
        gt_ps = [
            ctx.enter_context(nc.psum_tensor(f"gt{i}", [128, N_GRAPHS, D], f32))
            for i in range(2)
        ]
        o3_ps = [
            ctx.enter_context(nc.psum_tensor(f"o3{i}", [128, D], f32)) for i in range(2)
        ]
        init_sem = ctx.enter_context(nc.semaphore("init_sem"))
        io = ctx.enter_context(nc.semaphore("io"))
        a_sem = ctx.enter_context(nc.semaphore("a_sem"))
        gather_sem = ctx.enter_context(nc.semaphore("gather_sem"))
        store_sem = ctx.enter_context(nc.semaphore("store_sem"))
        pe_g = ctx.enter_context(nc.semaphore("pe_g"))
        pe_proj = ctx.enter_context(nc.semaphore("pe_proj"))
        act_sem = ctx.enter_context(nc.semaphore("act_sem"))
        dve_sem = ctx.enter_context(nc.semaphore("dve_sem"))

        @block.sync
        def _(sync):
            sync.dma_start(idx_sb[:, :], idx_d[:, :]).then_inc(io, 16)
            sync.dma_start(wp_sb[:, :], wp_d[:, :]).then_inc(io, 16)
            sync.dma_start(bias_sb[:, :], bias_d[:, :]).then_inc(io, 16)
            for b in range(NBl):
                cb = int(C_b[b])
                if b >= NBUF_A:
                    # A buffer reuse: PE done with block b-NBUF_A
                    sync.wait_ge(pe_g, 4 * (b - NBUF_A) + 4)
                sync.dma_start(
                    a_bufs[b % NBUF_A][:, : cb * 128],
                    a_d[:, int(off[b]) * 128 : int(off[b] + cb) * 128],
                ).then_inc(a_sem, 16)
                if b >= 2:
                    sb = b - 2  # store block b-2
                    sync.wait_ge(dve_sem, sb + 1)
                    sync.dma_start(
                        out_d[BLOCK * sb : BLOCK * sb + row_cnt[sb], :],
                        stage[: row_cnt[sb], (sb % 2) * D : (sb % 2) * D + D],
                    ).then_inc(store_sem, 16)
            for sb in (NBl - 2, NBl - 1):
                sync.wait_ge(dve_sem, sb + 1)
                sync.dma_start(
                    out_d[BLOCK * sb : BLOCK * sb + row_cnt[sb], :],
                    stage[: row_cnt[sb], (sb % 2) * D : (sb % 2) * D + D],
                ).then_inc(store_sem, 16)

        @block.gpsimd
        def _(gpsimd):
            gpsimd.load_library(mlp)
            gpsimd.wait_ge(io, 16)  # idx array resident (first io DMA)
            gpsimd.wait_ge(init_sem, NBUF)  # G buffers zeroed
            for b in range(NBl):
                nib = int(ni[b])
                nslots = (nib + 127) // 128
                if b >= NBUF:
                    gpsimd.wait_ge(pe_g, 4 * (b - NBUF) + 4)
                gpsimd.dma_gather(
                    g_bufs[b % NBUF][:, :nslots, :],
                    x_d[32768:, :],
                    idx_sb[:, int(off[b]) * 8 : int(off[b]) * 8 + nslots * 8],
                    nslots * 128,
                    nib,
                    D,
                    single_packet=False,
                    queue_num=b % 4,
                ).then_inc(gather_sem, 16)

        @block.tensor
        def _(tensor):
            tensor.wait_ge(io, 48)
            for b in range(NBl):
                tensor.wait_ge(gather_sem, 16 * (b + 1))
                tensor.wait_ge(a_sem, 16 * (b + 1))
                if b >= 2:
                    tensor.wait_ge(dve_sem, b - 1)  # o3 psum reuse
                gbuf = g_bufs[b % NBUF]
                abuf = a_bufs[b % NBUF_A]
                for l in range(N_GRAPHS):
                    cl = int(C[b, l])
                    for i in range(cl):
                        ci = int(L[b, l, i])
                        mm = tensor.matmul(
                            gt_ps[b % 2][:, l, :],
                            gbuf[:, ci, :],
                            abuf[:, ci * 128 : (ci + 1) * 128],
                            start=(i == 0),
                            stop=(i == cl - 1),
                        )
                    mm.then_inc(pe_g, 1)
                for l in range(N_GRAPHS):
                    tensor.wait_ge(act_sem, 4 * b + l + 1)
                    tensor.matmul(
                        o3_ps[b % 2][:, :],
                        gt_sb[:, ((b % 2) * N_GRAPHS + l) * D : ((b % 2) * N_GRAPHS + l + 1) * D],
                        wp_sb[:, l * D : (l + 1) * D],
                        start=(l == 0),
                        stop=(l == N_GRAPHS - 1),
                    ).then_inc(pe_proj, 1)

        @block.scalar
        def _(scalar):
            for i in range(NBUF):
                scalar.memzero(g_bufs[i][:, :, :]).then_inc(init_sem, 1)
            for b in range(NBl):
                for l in range(N_GRAPHS):
                    scalar.wait_ge(pe_g, 4 * b + 4)  # whole gt bank written
                    if b >= 2:
                        scalar.wait_ge(pe_proj, 4 * (b - 2) + l + 1)  # gt_sb reuse
                    scalar.copy(
                        gt_sb[:, ((b % 2) * N_GRAPHS + l) * D : ((b % 2) * N_GRAPHS + l + 1) * D],
                        gt_ps[b % 2][:, l, :],
                    ).then_inc(act_sem, 1)

        @block.vector
        def _(vector):
            for b in range(NBl):
                vector.wait_ge(pe_proj, 4 * b + 4)
                if b >= 2:
                    vector.wait_ge(store_sem, 16 * (b - 1))  # stage reuse
                vector.tensor_add(
                    stage[:, (b % 2) * D : (b % 2) * D + D],
                    o3_ps[b % 2][:, :],
                    bias_sb[:, :],
                ).then_inc(dve_sem, 1)

    nc.compile()
    return nc


_TRACE = {"on": False, "last": None}


def kernel(x, edge_rows, edge_cols, edge_vals, W, mixing_weight, bias):
    from concourse.bass_utils import run_bass_kernel_spmd

    sched = _host_schedule(edge_rows, edge_cols, edge_vals)
    nc = _build_nc(sched["C"], sched["C_b"], sched["total_chunks"], sched["L"], sched["ni"])

    x_bf16 = np.asarray(x, dtype=np.float32).astype(ml_dtypes.bfloat16)
    Wp = (np.asarray(mixing_weight, dtype=np.float32)[:, 0, None, None]
          * np.asarray(W, dtype=np.float32))  # [4,128,128]
    wp_arr = np.ascontiguousarray(
        np.transpose(Wp, (1, 0, 2)).reshape(D, N_GRAPHS * D)
    ).astype(ml_dtypes.bfloat16)
    bias_rep = np.ascontiguousarray(
        np.broadcast_to(np.asarray(bias, dtype=np.float32), (128, D))
    )

    in_maps = [
        {
            "x": x_bf16,
            "idxs": sched["idx_arrs"][s],
            "amat": sched["a_arrs"][s],
            "wp": wp_arr,
            "biasr": bias_rep,
        }
        for s in range(N_CORES)
    ]

    res = run_bass_kernel_spmd(
        nc, in_maps, core_ids=list(range(N_CORES)), trace=_TRACE["on"]
    )
    _TRACE["last"] = res
    out = np.concatenate(
        [np.asarray(res.results[s]["out"], dtype=np.float32) for s in range(N_CORES)],
        axis=0,
    )
    return out


# revision 15
# speedup vs baseline: 2.7614x; 1.0039x over previous
"""AdaptiveGraphConvolution on 8 TRN2 NeuronCores.

Math: out = sum_l m_l * segment_sum_l(val * (x @ W_l) gathered by col) + bias
Reordered: aggregate in input-feature space first (per graph), project after:
    g_l[r, :] = sum_{e in graph l, row_e = r} val_e * x[col_e, :]
    out[r, :] = sum_l g_l[r, :] @ (m_l * W_l) + bias

Sharding: destination rows across 8 cores (6250 rows each). Per core,
dest rows processed in 49 blocks of 128 rows. Edges of a block are packed
into 128-edge chunks (graph-pure). Per chunk:
  - dma_gather fetches the 128 source rows x[col] (bf16, 256B each) from HBM
  - a host-prebuilt "assignment" matrix A [128 edge, 128 destrow] bf16 with
    A[e, loc_e] = val_e is streamed from HBM
  - TensorE: gT_psum[l] += G_chunk^T @ A_chunk   ([feat, row] accumulation)
Then per block: ACT copies gT psum->SBUF (bf16), TensorE projects
out3 += gT_l^T @ W'_l (row-major), DVE adds bias, sync DMA stores.

idx trick: gather indices are int16; cols up to 49999 exceed 32767, so the
gather base is x[32768] and idx = col - 32768 (hardware treats idx as signed;
verified on silicon).
"""

import math
import numpy as np
import ml_dtypes

N_NODES = 50000
N_GRAPHS = 4
N_EDGES = 800000
D = 128
N_CORES = 8
ROWS_PER_CORE = N_NODES // N_CORES  # 6250
BLOCK = 128
NB = math.ceil(ROWS_PER_CORE / BLOCK)  # 49
NBUF = 3  # G buffering (>= in-flight gathers for multi-queue gen overlap)
NBUF_A = 3  # A-slab prefetch depth


def _host_schedule(edge_rows, edge_cols, edge_vals):
    """Build the SPMD-uniform chunk schedule + per-core idx/A arrays."""
    rows = np.asarray(edge_rows).astype(np.int64).ravel()  # [4*800000] graph-major
    cols = np.asarray(edge_cols).astype(np.int64).ravel()
    vals = np.asarray(edge_vals, dtype=np.float32).ravel()
    graph = np.repeat(np.arange(N_GRAPHS, dtype=np.int64), N_EDGES)

    core = rows // ROWS_PER_CORE
    local = rows - core * ROWS_PER_CORE
    blk = local // BLOCK
    loc = local % BLOCK

    # group key: (core, block, graph); count per group
    gkey = (core * NB + blk) * N_GRAPHS + graph
    n_groups = N_CORES * NB * N_GRAPHS
    cnt = np.bincount(gkey, minlength=n_groups).reshape(N_CORES, NB, N_GRAPHS)

    # uniform chunk counts across cores
    C = np.maximum(1, np.ceil(cnt.max(axis=0) / 128).astype(np.int64))  # [NB, 4]
    C_b = C.sum(axis=1)  # chunks per block
    total_chunks = int(C.sum())
    off = np.zeros(NB + 1, dtype=np.int64)
    off[1:] = np.cumsum(C_b)

    # Round-robin chunk order within each block: (l0 j0, l1 j0, ..., l0 j1, ...)
    # so cores' fill (low j first) concentrates padding at the call tail.
    Jmax = int(C.max())
    L = np.full((NB, N_GRAPHS, Jmax), -1, dtype=np.int64)  # (b,l,j) -> rr pos in block
    for b in range(NB):
        p = 0
        for j in range(int(C[b].max())):
            for l in range(N_GRAPHS):
                if j < C[b, l]:
                    L[b, l, j] = p
                    p += 1

    # rank of each edge within its (core, block, graph) group
    order = np.argsort(gkey, kind="stable")
    sorted_key = gkey[order]
    grp_start = np.searchsorted(sorted_key, np.arange(n_groups), side="left")
    rank_sorted = np.arange(len(order)) - grp_start[sorted_key]
    rank = np.empty_like(rank_sorted)
    rank[order] = rank_sorted

    chunk_in_run = rank // 128
    slot = rank % 128
    chunk = off[blk] + L[blk, graph, chunk_in_run]  # global chunk id (rr order)
    pos = chunk * 128 + slot  # position in the core's edge stream

    # per-call transferred idx count: cover every core's last real edge
    pos_in_call = pos - off[blk] * 128
    ni = np.zeros(NB, dtype=np.int64)
    np.maximum.at(ni, blk, pos_in_call + 1)
    ni = np.minimum(((ni + 15) // 16) * 16, C_b * 128).astype(np.int64)

    total_idx = total_chunks * 128
    idx_arrs, a_arrs = [], []
    for s in range(N_CORES):
        m = core == s
        idx_flat = np.zeros(total_idx, dtype=np.int16)
        idx_flat[pos[m]] = (cols[m] - 32768).astype(np.int16)
        wrapped = idx_flat.reshape(-1, 16).T  # [16, total_idx/16]
        idx_arrs.append(np.tile(wrapped, (8, 1)).copy())

        A = np.zeros((128, total_chunks, 128), dtype=ml_dtypes.bfloat16)
        A[slot[m], chunk[m], loc[m]] = vals[m].astype(ml_dtypes.bfloat16)
        a_arrs.append(A.reshape(128, total_chunks * 128))

    return {
        "C": C,
        "C_b": C_b,
        "L": L,
        "ni": ni,
        "total_chunks": total_chunks,
        "idx_arrs": idx_arrs,
        "a_arrs": a_arrs,
    }


def _build_nc(C, C_b, total_chunks, L, ni):
    import concourse.bacc as bacc
    import concourse.bass as bass
    import concourse.mybir as mybir
    from concourse.library_config import mlp
    import contextlib

    Cmax = int(C_b.max())
    total8 = total_chunks * 8
    NBl = NB
    # offsets per block, in chunks
    off = np.zeros(NBl + 1, dtype=np.int64)
    off[1:] = np.cumsum(C_b)
    row_cnt = [min(BLOCK, ROWS_PER_CORE - BLOCK * b) for b in range(NBl)]

    nc = bacc.Bacc("TRN2", dynamic_dma_scratch_size=32768, num_swdge_queues=4)
    bf16 = mybir.dt.bfloat16
    f32 = mybir.dt.float32

    x_d = nc.declare_dram_parameter("x", [N_NODES, D], bf16, isOutput=False)
    idx_d = nc.declare_dram_parameter("idxs", [128, total8], mybir.dt.int16, isOutput=False)
    a_d = nc.declare_dram_parameter("amat", [128, total_chunks * 128], bf16, isOutput=False)
    wp_d = nc.declare_dram_parameter("wp", [128, N_GRAPHS * D], bf16, isOutput=False)
    bias_d = nc.declare_dram_parameter("biasr", [128, D], f32, isOutput=False)
    out_d = nc.declare_dram_parameter("out", [ROWS_PER_CORE, D], f32, isOutput=True)

    with contextlib.ExitStack() as ctx:
        block = ctx.enter_context(nc.Block())
        idx_sb = ctx.enter_context(nc.sbuf_tensor("idx_sb", [128, total8], mybir.dt.int16))
        g_bufs = [
            ctx.enter_context(nc.sbuf_tensor(f"g{i}", [128, Cmax, D], bf16))
            for i in range(NBUF)
        ]
        a_bufs = [
            ctx.enter_context(nc.sbuf_tensor(f"a{i}", [128, Cmax * 128], bf16))
            for i in range(NBUF_A)
        ]
        wp_sb = ctx.enter_context(nc.sbuf_tensor("wp_sb", [128, N_GRAPHS * D], bf16))
        bias_sb = ctx.enter_context(nc.sbuf_tensor("bias_sb", [128, D], f32))
        gt_sb = ctx.enter_context(nc.sbuf_tensor("gt_sb", [128, 2 * N_GRAPHS * D], bf16))
        stage = ctx.enter_context(nc.sbuf_tensor("stage", [128, 2 * D], f32))
        gt_ps = [
            ctx.enter_context(nc.psum_tensor(f"gt{i}", [128, N_GRAPHS, D], f32))
            for i in range(2)
        ]
        o3_ps = [
            ctx.enter_context(nc.psum_tensor(f"o3{i}", [128, D], f32)) for i in range(2)
        ]
        init_sem = ctx.enter_context(nc.semaphore("init_sem"))
        io = ctx.enter_context(nc.semaphore("io"))
        a_sem = ctx.enter_context(nc.semaphore("a_sem"))
        gather_sem = ctx.enter_context(nc.semaphore("gather_sem"))
        store_sem = ctx.enter_context(nc.semaphore("store_sem"))
        pe_g = ctx.enter_context(nc.semaphore("pe_g"))
        pe_proj = ctx.enter_context(nc.semaphore("pe_proj"))
        act_sem = ctx.enter_context(nc.semaphore("act_sem"))
        dve_sem = ctx.enter_context(nc.semaphore("dve_sem"))

        @block.sync
        def _(sync):
            sync.dma_start(idx_sb[:, :], idx_d[:, :]).then_inc(io, 16)
            sync.dma_start(wp_sb[:, :], wp_d[:, :]).then_inc(io, 16)
            sync.dma_start(bias_sb[:, :], bias_d[:, :]).then_inc(io, 16)
            for b in range(NBl):
                cb = int(C_b[b])
                if b >= NBUF_A:
                    # A buffer reuse: PE done with block b-NBUF_A
                    sync.wait_ge(pe_g, 4 * (b - NBUF_A) + 4)
                sync.dma_start(
                    a_bufs[b % NBUF_A][:, : cb * 128],
                    a_d[:, int(off[b]) * 128 : int(off[b] + cb) * 128],
                ).then_inc(a_sem, 16)
                if b >= 2:
                    sb = b - 2  # store block b-2
                    sync.wait_ge(dve_sem, sb + 1)
                    sync.dma_start(
                        out_d[BLOCK * sb : BLOCK * sb + row_cnt[sb], :],
                        stage[: row_cnt[sb], (sb % 2) * D : (sb % 2) * D + D],
                    ).then_inc(store_sem, 16)
            for sb in (NBl - 2, NBl - 1):
                sync.wait_ge(dve_sem, sb + 1)
                sync.dma_start(
                    out_d[BLOCK * sb : BLOCK * sb + row_cnt[sb], :],
                    stage[: row_cnt[sb], (sb % 2) * D : (sb % 2) * D + D],
                ).then_inc(store_sem, 16)

        @block.gpsimd
        def _(gpsimd):
            gpsimd.load_library(mlp)
            gpsimd.wait_ge(io, 16)  # idx array resident (first io DMA)
            gpsimd.wait_ge(init_sem, NBUF)  # G buffers zeroed
            for b in range(NBl):
                nib = int(ni[b])
                nslots = (nib + 127) // 128
                if b >= NBUF:
                    gpsimd.wait_ge(pe_g, 4 * (b - NBUF) + 4)
                gpsimd.dma_gather(
                    g_bufs[b % NBUF][:, :nslots, :],
                    x_d[32768:, :],
                    idx_sb[:, int(off[b]) * 8 : int(off[b]) * 8 + nslots * 8],
                    nslots * 128,
                    nib,
                    D,
                    single_packet=False,
                    queue_num=b % 4,
                ).then_inc(gather_sem, 16)

        @block.tensor
        def _(tensor):
            tensor.wait_ge(io, 48)
            for b in range(NBl):
                tensor.wait_ge(gather_sem, 16 * (b + 1))
                tensor.wait_ge(a_sem, 16 * (b + 1))
                if b >= 2:
                    tensor.wait_ge(dve_sem, b - 1)  # o3 psum reuse
                gbuf = g_bufs[b % NBUF]
                abuf = a_bufs[b % NBUF_A]
                for l in range(N_GRAPHS):
                    cl = int(C[b, l])
                    for i in range(cl):
                        ci = int(L[b, l, i])
                        mm = tensor.matmul(
                            gt_ps[b % 2][:, l, :],
                            gbuf[:, ci, :],
                            abuf[:, ci * 128 : (ci + 1) * 128],
                            start=(i == 0),
                            stop=(i == cl - 1),
                        )
                    mm.then_inc(pe_g, 1)
                for l in range(N_GRAPHS):
                    tensor.wait_ge(act_sem, 4 * b + l + 1)
                    tensor.matmul(
                        o3_ps[b % 2][:, :],
                        gt_sb[:, ((b % 2) * N_GRAPHS + l) * D : ((b % 2) * N_GRAPHS + l + 1) * D],
                        wp_sb[:, l * D : (l + 1) * D],
                        start=(l == 0),
                        stop=(l == N_GRAPHS - 1),
                    ).then_inc(pe_proj, 1)

        @block.scalar
        def _(scalar):
            for i in range(NBUF):
                scalar.memzero(g_bufs[i][:, :, :]).then_inc(init_sem, 1)
            for b in range(NBl):
                for l in range(N_GRAPHS):
                    scalar.wait_ge(pe_g, 4 * b + 4)  # whole gt bank written
                    if b >= 2:
                        scalar.wait_ge(pe_proj, 4 * (b - 2) + l + 1)  # gt_sb reuse
                    scalar.copy(
                        gt_sb[:, ((b % 2) * N_GRAPHS + l) * D : ((b % 2) * N_GRAPHS + l + 1) * D],
                        gt_ps[b % 2][:, l, :],
                    ).then_inc(act_sem, 1)

        @block.vector
        def _(vector):
            for b in range(NBl):
                vector.wait_ge(pe_proj, 4 * b + 4)
                if b >= 2:
                    vector.wait_ge(store_sem, 16 * (b - 1))  # stage reuse
                vector.tensor_add(
                    stage[:, (b % 2) * D : (b % 2) * D + D],
                    o3_ps[b % 2][:, :],
                    bias_sb[:, :],
                ).then_inc(dve_sem, 1)

    nc.compile()
    return nc


_TRACE = {"on": False, "last": None}


def kernel(x, edge_rows, edge_cols, edge_vals, W, mixing_weight, bias):
    from concourse.bass_utils import run_bass_kernel_spmd

    sched = _host_schedule(edge_rows, edge_cols, edge_vals)
    nc = _build_nc(sched["C"], sched["C_b"], sched["total_chunks"], sched["L"], sched["ni"])

    x_bf16 = np.asarray(x, dtype=np.float32).astype(ml_dtypes.bfloat16)
    Wp = (np.asarray(mixing_weight, dtype=np.float32)[:, 0, None, None]
          * np.asarray(W, dtype=np.float32))  # [4,128,128]
    wp_arr = np.ascontiguousarray(
        np.transpose(Wp, (1, 0, 2)).reshape(D, N_GRAPHS * D)
    ).astype(ml_dtypes.bfloat16)
    bias_rep = np.ascontiguousarray(
        np.broadcast_to(np.asarray(bias, dtype=np.float32), (128, D))
    )

    in_maps = [
        {
            "x": x_bf16,
            "idxs": sched["idx_arrs"][s],
            "amat": sched["a_arrs"][s],
            "wp": wp_arr,
            "biasr": bias_rep,
        }
        for s in range(N_CORES)
    ]

    res = run_bass_kernel_spmd(
        nc, in_maps, core_ids=list(range(N_CORES)), trace=_TRACE["on"]
    )
    _TRACE["last"] = res
    out = np.concatenate(
        [np.asarray(res.results[s]["out"], dtype=np.float32) for s in range(N_CORES)],
        axis=0,
    )
    return out
